# revision 1
# baseline (speedup 1.0000x reference)
"""Single-head attention (InterModalAttention) Bass kernel for 8 TRN2 cores.

Sharding: batch (4) x query-half (2) -> 8 cores. Each core computes K/V for
its batch element (full 2048-seq) and attention for its 1024 queries.

Layout strategy (all matmuls contract over the partition dim):
  - Host pre-transposes x and weights -> xT [d,s], WT [d,e] so no on-chip
    transpose of inputs is needed.
  - qT/kT computed as [e, s] tiles (lhsT=WT tile, rhs=xT tile); bias added
    per-partition during PSUM->SBUF copyback.
  - v computed natural [s, e] (lhsT=xT tile, rhs=WvT tile); bias bv folded
    into the final epilogue (softmax rows sum to 1).
  - scores[i,j] psum accumulated over 8 e-tiles; exp on ACT engine with
    scale=1/32 and accum_out giving row-sums for free.
  - attn tiles PE-transposed (128x128) -> lhsT for out = attnT.T @ v,
    accumulated over 16 j-tiles in PSUM.
  - epilogue: out = psum * (1/rowsum) + bv.
All matmul operands use float32r (full-rate fp32 on the PE at N>=512).
"""
import sys
import numpy as np

for p in ("/opt/trn_rl_repo",):
    if p not in sys.path:
        sys.path.insert(0, p)

B, S, D = 4, 2048, 1024
NQ = 1024          # queries per core
NCORES = 8
P = 128
INV_SQRT_D = 1.0 / 32.0

_CACHE = {}


def build_nc():
    from contextlib import ExitStack
    import concourse.mybir as mybir
    import concourse.tile as tile
    from concourse import bacc
    from concourse.masks import make_identity

    F32 = mybir.dt.float32
    FR = mybir.dt.float32r
    AF = mybir.ActivationFunctionType

    nc = bacc.Bacc("TRN2", debug=False)

    xkvT = nc.dram_tensor("xkvT", (D, S), FR, kind="ExternalInput")
    xqT = nc.dram_tensor("xqT", (D, NQ), FR, kind="ExternalInput")
    wqT = nc.dram_tensor("wqT", (D, D), FR, kind="ExternalInput")
    wkT = nc.dram_tensor("wkT", (D, D), FR, kind="ExternalInput")
    wvT = nc.dram_tensor("wvT", (D, D), FR, kind="ExternalInput")
    bq = nc.dram_tensor("bq", (D,), F32, kind="ExternalInput")
    bk = nc.dram_tensor("bk", (D,), F32, kind="ExternalInput")
    bv = nc.dram_tensor("bv", (D,), F32, kind="ExternalInput")
    out = nc.dram_tensor("out", (NQ, D), F32, kind="ExternalOutput")

    ET = D // P            # 8 e-tiles
    DT = D // P            # 8 d-tiles
    SC = S // 512          # 4 s-chunks
    SB = S // P            # 16 s-blocks (j-tiles)
    IG = NQ // 512         # 2 i-groups
    EC = D // 512          # 2 e-chunks

    with tile.TileContext(nc) as tc, ExitStack() as ctx:
        consts = ctx.enter_context(tc.tile_pool(name="consts", bufs=1))
        ps512 = ctx.enter_context(tc.tile_pool(name="ps512", bufs=2, space="PSUM"))
        outps = ctx.enter_context(tc.tile_pool(name="outps", bufs=2, space="PSUM"))
        tpps = ctx.enter_context(tc.tile_pool(name="tpps", bufs=2, space="PSUM"))
        dram = ctx.enter_context(tc.tile_pool(name="dram", bufs=1, space="DRAM"))

        _eng = [nc.sync, nc.gpsimd, nc.scalar]
        _dmac = [0]
        def dma(out_ap, in_ap):
            e = _eng[_dmac[0] % len(_eng)]
            _dmac[0] += 1
            e.dma_start(out_ap, in_ap)

        # ---- constants ----
        ident_f = consts.tile([P, P], F32)
        make_identity(nc, ident_f)
        ident = consts.tile([P, P], FR)
        nc.gpsimd.dma_start(ident[:], ident_f[:])

        ones_f = consts.tile([1, P], F32)
        nc.gpsimd.memset(ones_f[:], 1.0)
        ones = consts.tile([1, P], FR)
        nc.gpsimd.dma_start(ones[:], ones_f[:])

        bv_sb = consts.tile([1, D], FR)
        nc.gpsimd.dma_start(bv_sb[:], bv[:].rearrange("(one d) -> one d", one=1))
        bq_sb = consts.tile([P, ET], F32)
        nc.sync.dma_start(bq_sb[:], bq[:].rearrange("(t p) -> p t", p=P))
        bk_sb = consts.tile([P, ET], F32)
        nc.sync.dma_start(bk_sb[:], bk[:].rearrange("(t p) -> p t", p=P))

        # bv broadcast to [P, D] via ones.T @ bv (K=1 matmul)
        bv_bcast = consts.tile([P, D], F32)
        for ec in range(EC):
            pstmp = ps512.tile([P, 512], F32, tag="ps512")
            nc.tensor.matmul(pstmp[:], ones[:], bv_sb[:, ec * 512:(ec + 1) * 512],
                             start=True, stop=True)
            nc.any.tensor_copy(bv_bcast[:, ec * 512:(ec + 1) * 512], pstmp[:])

        qT_dram = dram.tile([D, NQ], FR)
        kpool = ctx.enter_context(tc.tile_pool(name="kpool", bufs=1))
        kT = kpool.tile([P, ET, S], FR)      # [e-part, e-tile, j]

        # ---- Phase 1: Q projection (wk prefetched) ----
        wk_ctx = tc.tile_pool(name="wk", bufs=1)
        wkp = wk_ctx.__enter__()
        with tc.tile_pool(name="wq", bufs=1) as wqp, \
             tc.tile_pool(name="xq", bufs=1) as xqp, \
             tc.tile_pool(name="qo", bufs=2) as qop:
            wq_sb = wqp.tile([P, DT, D], FR)
            for dt in range(DT):
                dma(wq_sb[:, dt, :], wqT[dt * P:(dt + 1) * P, :])
            wk_sb = wkp.tile([P, DT, D], FR)
            for dt in range(DT):
                dma(wk_sb[:, dt, :], wkT[dt * P:(dt + 1) * P, :])
            for g in range(IG):
                xq_g = xqp.tile([P, DT, 512], FR, tag="xq")
                for dt in range(DT):
                    dma(xq_g[:, dt, :],
                                      xqT[dt * P:(dt + 1) * P, g * 512:(g + 1) * 512])
                for et in range(ET):
                    psq = ps512.tile([P, 512], F32, tag="ps512")
                    for dt in range(DT):
                        nc.tensor.matmul(psq[:], wq_sb[:, dt, et * P:(et + 1) * P],
                                         xq_g[:, dt, :], start=(dt == 0), stop=(dt == DT - 1))
                    qo = qop.tile([P, 512], FR, tag="qo")
                    nc.vector.tensor_scalar_add(qo[:], psq[:], bq_sb[:, et:et + 1])
                    dma(qT_dram[et * P:(et + 1) * P, g * 512:(g + 1) * 512], qo[:])

        # ---- Phase 2: K projection -> kT resident [e, j] ----
        with tc.tile_pool(name="xk", bufs=2) as xkp:
            for sc in range(SC):
                xk_g = xkp.tile([P, DT, 512], FR, tag="xk")
                for dt in range(DT):
                    dma(xk_g[:, dt, :],
                                      xkvT[dt * P:(dt + 1) * P, sc * 512:(sc + 1) * 512])
                for et in range(ET):
                    psk = ps512.tile([P, 512], F32, tag="ps512")
                    for dt in range(DT):
                        nc.tensor.matmul(psk[:], wk_sb[:, dt, et * P:(et + 1) * P],
                                         xk_g[:, dt, :], start=(dt == 0), stop=(dt == DT - 1))
                    nc.vector.tensor_scalar_add(kT[:, et, sc * 512:(sc + 1) * 512],
                                                psk[:], bk_sb[:, et:et + 1])

        # ---- Phase 3: V projection -> v resident [j, e] (no bias) ----
        wk_ctx.__exit__(None, None, None)
        vpool = ctx.enter_context(tc.tile_pool(name="vpool", bufs=1))
        vN = vpool.tile([P, SB, D], FR)      # [s-part, j-tile, e]
        with tc.tile_pool(name="wv", bufs=1) as wvp, \
             tc.tile_pool(name="xv", bufs=2) as xvp:
            wv_sb = wvp.tile([P, DT, D], FR)
            for dt in range(DT):
                dma(wv_sb[:, dt, :], wvT[dt * P:(dt + 1) * P, :])
            for sb_i in range(SB):
                xv_g = xvp.tile([P, DT, P], FR, tag="xv")
                for dt in range(DT):
                    dma(xv_g[:, dt, :],
                                      xkvT[dt * P:(dt + 1) * P, sb_i * P:(sb_i + 1) * P])
                for ec in range(EC):
                    psv = ps512.tile([P, 512], F32, tag="ps512")
                    for dt in range(DT):
                        nc.tensor.matmul(psv[:], xv_g[:, dt, :],
                                         wv_sb[:, dt, ec * 512:(ec + 1) * 512],
                                         start=(dt == 0), stop=(dt == DT - 1))
                    nc.any.tensor_copy(vN[:, sb_i, ec * 512:(ec + 1) * 512], psv[:])

        # ---- Phase 4: attention ----
        with tc.tile_pool(name="qg", bufs=1) as qgp, \
             tc.tile_pool(name="attn", bufs=3) as attnp, \
             tc.tile_pool(name="attnT", bufs=6) as attnTp, \
             tc.tile_pool(name="epi", bufs=2) as epip:
            for g in range(IG):
                qT_g = qgp.tile([P, ET, 512], FR, tag="qg")
                for et in range(ET):
                    dma(qT_g[:, et, :],
                                      qT_dram[et * P:(et + 1) * P, g * 512:(g + 1) * 512])
                for ib in range(4):
                    i0 = ib * P
                    out_ps = [outps.tile([P, 512], F32, tag=f"outps{ec}", name=f"out_ps{ec}")
                              for ec in range(EC)]
                    rs = epip.tile([P, SC], F32, tag="rs")
                    for jc in range(SC):
                        sc_ps = ps512.tile([P, 512], F32, tag="ps512")
                        for et in range(ET):
                            nc.tensor.matmul(sc_ps[:], qT_g[:, et, i0:i0 + P],
                                             kT[:, et, jc * 512:(jc + 1) * 512],
                                             start=(et == 0), stop=(et == ET - 1))
                        attn = attnp.tile([P, 512], FR, tag="attn")
                        nc.scalar.activation(attn[:], sc_ps[:], AF.Exp,
                                             scale=INV_SQRT_D, accum_out=rs[:, jc:jc + 1])
                        for jt in range(4):
                            jg = jc * 4 + jt
                            tps = tpps.tile([P, P], FR, tag="tps")
                            nc.tensor.transpose(tps[:], attn[:, jt * P:(jt + 1) * P], ident[:])
                            attnT = attnTp.tile([P, P], FR, tag="attnT")
                            nc.any.tensor_copy(attnT[:], tps[:])
                            for ec in range(EC):
                                nc.tensor.matmul(out_ps[ec][:], attnT[:],
                                                 vN[:, jg, ec * 512:(ec + 1) * 512],
                                                 start=(jg == 0), stop=(jg == SB - 1))
                    rsum = epip.tile([P, 1], F32, tag="rsum")
                    nc.vector.tensor_reduce(rsum[:], rs[:], mybir.AxisListType.X,
                                            mybir.AluOpType.add)
                    invs = epip.tile([P, 1], F32, tag="invs")
                    nc.vector.reciprocal(invs[:], rsum[:])
                    out_sb = epip.tile([P, D], F32, tag="out_sb")
                    for ec in range(EC):
                        nc.vector.tensor_scalar_mul(out_sb[:, ec * 512:(ec + 1) * 512],
                                                    out_ps[ec][:], invs[:])
                    nc.vector.tensor_add(out_sb[:], out_sb[:], bv_bcast[:])
                    r0 = g * 512 + i0
                    dma(out[r0:r0 + P, :], out_sb[:])

    nc.compile()
    return nc


def make_in_maps(x, Wq, bq, Wk, bk, Wv, bv):
    x = np.asarray(x, np.float32)
    wqT = np.ascontiguousarray(np.asarray(Wq, np.float32).T)
    wkT = np.ascontiguousarray(np.asarray(Wk, np.float32).T)
    wvT = np.ascontiguousarray(np.asarray(Wv, np.float32).T)
    bq = np.ascontiguousarray(np.asarray(bq, np.float32))
    bk = np.ascontiguousarray(np.asarray(bk, np.float32))
    bv = np.ascontiguousarray(np.asarray(bv, np.float32))
    in_maps = []
    for c in range(NCORES):
        b, h = c // 2, c % 2
        xb = x[b]
        in_maps.append({
            "xkvT": np.ascontiguousarray(xb.T),
            "xqT": np.ascontiguousarray(xb[h * NQ:(h + 1) * NQ].T),
            "wqT": wqT, "wkT": wkT, "wvT": wvT,
            "bq": bq, "bk": bk, "bv": bv,
        })
    return in_maps


def get_nc():
    if "nc" not in _CACHE:
        _CACHE["nc"] = build_nc()
    return _CACHE["nc"]


def kernel(x, Wq, bq, Wk, bk, Wv, bv):
    from concourse.bass_utils import run_bass_kernel_spmd
    nc = get_nc()
    in_maps = make_in_maps(x, Wq, bq, Wk, bk, Wv, bv)
    res = run_bass_kernel_spmd(nc, in_maps, core_ids=list(range(NCORES)))
    out = np.empty((B, S, D), np.float32)
    for c in range(NCORES):
        b, h = c // 2, c % 2
        out[b, h * NQ:(h + 1) * NQ] = res.results[c]["out"]
    return out



# revision 4
# speedup vs baseline: 1.3520x; 1.3520x over previous
"""Single-head attention (InterModalAttention) Bass kernel for 8 TRN2 cores.

Sharding: batch (4) x query-half (2) -> 8 cores. Each core computes K/V for
its batch element (full 2048-seq) and attention for its 1024 queries.

v2 layout strategy (all matmuls contract over the partition dim, all matmul
operands bf16 -> full PE rate, half the DMA/SBUF of fp32):
  - Host pre-transposes x and weights and casts to bf16: xT [d,s], WT [d,e].
  - qT/kT computed as [e, i/j] tiles (lhsT=WT tile, rhs=xT tile); bias added
    per-partition during the PSUM->SBUF copyback. qT stays SBUF-resident
    (no DRAM round-trip).
  - v computed natural [j, e] (lhsT=xT tile, rhs=WvT tile); bv folded into
    the epilogue (softmax rows sum to 1). K and V projections share one
    streaming pass over xT.
  - scores computed TRANSPOSED [j, i] (lhsT=kT tile, rhs=qT tile) so the
    exp output is directly the lhsT of the attn@V matmul - no PE transposes.
  - rowsums as tiny attnT.T @ ones matmuls accumulating into PSUM [i, 1]
    columns, which is exactly the per-partition orientation the epilogue
    division needs.
  - epilogue: out = out_psum * (1/rowsum) + bv_bcast.
"""
import sys
import numpy as np

for p in ("/opt/trn_rl_repo",):
    if p not in sys.path:
        sys.path.insert(0, p)

import ml_dtypes

B, S, D = 4, 2048, 1024
NQ = 1024          # queries per core
NCORES = 8
P = 128
INV_SQRT_D = 1.0 / 32.0
BF16 = ml_dtypes.bfloat16

_CACHE = {}


def build_nc():
    from contextlib import ExitStack
    import concourse.mybir as mybir
    import concourse.tile as tile
    from concourse import bacc

    F32 = mybir.dt.float32
    BF = mybir.dt.bfloat16
    AF = mybir.ActivationFunctionType

    nc = bacc.Bacc("TRN2", debug=False)

    xkvT = nc.dram_tensor("xkvT", (D, S), BF, kind="ExternalInput")
    xqT = nc.dram_tensor("xqT", (D, NQ), BF, kind="ExternalInput")
    wqT = nc.dram_tensor("wqT", (D, D), BF, kind="ExternalInput")
    wkT = nc.dram_tensor("wkT", (D, D), BF, kind="ExternalInput")
    wvT = nc.dram_tensor("wvT", (D, D), BF, kind="ExternalInput")
    bq = nc.dram_tensor("bq", (D,), F32, kind="ExternalInput")
    bk = nc.dram_tensor("bk", (D,), F32, kind="ExternalInput")
    bv = nc.dram_tensor("bv", (D,), F32, kind="ExternalInput")
    out = nc.dram_tensor("out", (NQ, D), F32, kind="ExternalOutput")

    ET = D // P            # 8 e-tiles
    DT = D // P            # 8 d-tiles
    SC = S // 512          # 4 x stream chunks
    SB = S // P            # 16 j-tiles
    IG = NQ // 512         # 2 i-groups
    EC = D // 512          # 2 e-chunks

    with tile.TileContext(nc) as tc, ExitStack() as ctx:
        consts = ctx.enter_context(tc.tile_pool(name="consts", bufs=1))
        ps512 = ctx.enter_context(tc.tile_pool(name="ps512", bufs=2, space="PSUM"))
        outps = ctx.enter_context(tc.tile_pool(name="outps", bufs=2, space="PSUM"))
        rsps = ctx.enter_context(tc.tile_pool(name="rsps", bufs=1, space="PSUM"))
        wpool = ctx.enter_context(tc.tile_pool(name="wpool", bufs=1))
        xqpool = ctx.enter_context(tc.tile_pool(name="xqpool", bufs=1))
        qpool = ctx.enter_context(tc.tile_pool(name="qpool", bufs=1))
        kpool = ctx.enter_context(tc.tile_pool(name="kpool", bufs=1))
        vpool = ctx.enter_context(tc.tile_pool(name="vpool", bufs=1))

        # ---- resident tensors ----
        wq_sb = wpool.tile([P, DT, D], BF)
        wk_sb = wpool.tile([P, DT, D], BF)
        wv_sb = wpool.tile([P, DT, D], BF)
        xq_sb = xqpool.tile([P, DT, NQ], BF)
        qT = qpool.tile([P, ET, NQ], BF)     # [e-part, e-tile, i]
        kT = kpool.tile([P, ET, S], BF)      # [e-part, e-tile, j]
        vN = vpool.tile([P, SB, D], BF)      # [j-part, j-tile, e]

        # ---- startup DMAs: wq on sync, xq on gpsimd in parallel so the
        # first Q-proj matmul can start as early as possible; everything
        # else behind them ----
        for dt in range(DT):
            nc.sync.dma_start(wq_sb[:, dt, :], wqT[dt * P:(dt + 1) * P, :])
            nc.gpsimd.dma_start(xq_sb[:, dt, :], xqT[dt * P:(dt + 1) * P, :])

        # constants on the scalar queue (tiny)
        bq_sb = consts.tile([P, ET], F32)
        nc.scalar.dma_start(bq_sb[:], bq[:].rearrange("(t p) -> p t", p=P))
        bk_sb = consts.tile([P, ET], F32)
        nc.scalar.dma_start(bk_sb[:], bk[:].rearrange("(t p) -> p t", p=P))
        bv_f = consts.tile([1, D], F32)
        nc.scalar.dma_start(bv_f[:], bv[:].rearrange("(one d) -> one d", one=1))
        ones_f = consts.tile([1, P], F32)
        nc.gpsimd.memset(ones_f[:], 1.0)
        ones_row = consts.tile([1, P], BF)
        nc.vector.tensor_copy(ones_row[:], ones_f[:])
        onesc_f = consts.tile([P, 1], F32)
        nc.gpsimd.memset(onesc_f[:], 1.0)
        ones_col = consts.tile([P, 1], BF)
        nc.vector.tensor_copy(ones_col[:], onesc_f[:])
        bv_bf = consts.tile([1, D], BF)
        nc.vector.tensor_copy(bv_bf[:], bv_f[:])

        # bv broadcast to [P, D] via ones.T @ bv (K=1 matmul)
        bv_bcast = consts.tile([P, D], F32)
        for ec in range(EC):
            pstmp = ps512.tile([P, 512], F32, tag="ps512")
            nc.tensor.matmul(pstmp[:], ones_row[:], bv_bf[:, ec * 512:(ec + 1) * 512],
                             start=True, stop=True)
            nc.any.tensor_copy(bv_bcast[:, ec * 512:(ec + 1) * 512], pstmp[:])

        # remaining weight loads (behind the startup DMAs, ahead of use)
        for dt in range(DT):
            nc.sync.dma_start(wk_sb[:, dt, :], wkT[dt * P:(dt + 1) * P, :])
        for dt in range(DT):
            nc.scalar.dma_start(wv_sb[:, dt, :], wvT[dt * P:(dt + 1) * P, :])

        # ---- Phase 1: Q projection -> qT resident [e, i] ----
        for g in range(IG):
            for et in range(ET):
                psq = ps512.tile([P, 512], F32, tag="ps512")
                for dt in range(DT):
                    nc.tensor.matmul(psq[:], wq_sb[:, dt, et * P:(et + 1) * P],
                                     xq_sb[:, dt, g * 512:(g + 1) * 512],
                                     start=(dt == 0), stop=(dt == DT - 1))
                nc.vector.tensor_scalar_add(qT[:, et, g * 512:(g + 1) * 512],
                                            psq[:], bq_sb[:, et:et + 1])

        # ---- Phase 2: K + V projections on one streaming pass of xkvT ----
        with tc.tile_pool(name="xkv", bufs=2) as xkvp:
            for sc in range(SC):
                xg = xkvp.tile([P, DT, 512], BF, tag="xkv")
                for dt in range(DT):
                    e = (nc.sync, nc.gpsimd, nc.scalar)[dt % 3]
                    e.dma_start(xg[:, dt, :],
                                xkvT[dt * P:(dt + 1) * P, sc * 512:(sc + 1) * 512])
                for et in range(ET):
                    psk = ps512.tile([P, 512], F32, tag="ps512")
                    for dt in range(DT):
                        nc.tensor.matmul(psk[:], wk_sb[:, dt, et * P:(et + 1) * P],
                                         xg[:, dt, :], start=(dt == 0), stop=(dt == DT - 1))
                    nc.vector.tensor_scalar_add(kT[:, et, sc * 512:(sc + 1) * 512],
                                                psk[:], bk_sb[:, et:et + 1])
                for j4 in range(4):
                    jg = sc * 4 + j4
                    for ec in range(EC):
                        psv = ps512.tile([P, 512], F32, tag="ps512")
                        for dt in range(DT):
                            nc.tensor.matmul(psv[:], xg[:, dt, j4 * P:(j4 + 1) * P],
                                             wv_sb[:, dt, ec * 512:(ec + 1) * 512],
                                             start=(dt == 0), stop=(dt == DT - 1))
                        nc.any.tensor_copy(vN[:, jg, ec * 512:(ec + 1) * 512], psv[:])

        # ---- Phase 3: attention ----
        with tc.tile_pool(name="attn", bufs=1) as attnp, \
             tc.tile_pool(name="epi", bufs=2) as epip:
            for g in range(IG):
                aT = attnp.tile([P, SB, 512], BF, tag="attn")  # [j-part, j-tile, i]
                # stride-2 columns: one sequential accumulation group per
                # column, 8-byte-aligned PSUM writes
                rs_ps = rsps.tile([P, 8], F32, tag="rs")       # [i-part, 2*i-block]
                for jt in range(SB):
                    sps = ps512.tile([P, 512], F32, tag="ps512")
                    for et in range(ET):
                        nc.tensor.matmul(sps[:], kT[:, et, jt * P:(jt + 1) * P],
                                         qT[:, et, g * 512:(g + 1) * 512],
                                         start=(et == 0), stop=(et == ET - 1))
                    nc.scalar.activation(aT[:, jt, :], sps[:], AF.Exp,
                                         scale=INV_SQRT_D)
                invT = epip.tile([P, 8], F32, tag="invT")
                for ib in range(4):
                    out_ps = [outps.tile([P, 512], F32, tag=f"outps{ec}", name=f"out_ps{ec}")
                              for ec in range(EC)]
                    for jg in range(SB):
                        for ec in range(EC):
                            nc.tensor.matmul(out_ps[ec][:],
                                             aT[:, jg, ib * P:(ib + 1) * P],
                                             vN[:, jg, ec * 512:(ec + 1) * 512],
                                             start=(jg == 0), stop=(jg == SB - 1))
                        # rowsum for this (ib, jg) chunk: reads the aT chunk
                        # the matmuls above just consumed, so exp is done;
                        # per-column group is sequential within the rs bank
                        nc.tensor.matmul(rs_ps[:, 2 * ib:2 * ib + 1],
                                         aT[:, jg, ib * P:(ib + 1) * P],
                                         ones_col[:],
                                         start=(jg == 0), stop=(jg == SB - 1))
                    nc.vector.reciprocal(invT[:, 2 * ib:2 * ib + 1],
                                         rs_ps[:, 2 * ib:2 * ib + 1])
                    out_sb = epip.tile([P, D], F32, tag="out_sb")
                    for ec in range(EC):
                        nc.vector.tensor_scalar_mul(out_sb[:, ec * 512:(ec + 1) * 512],
                                                    out_ps[ec][:], invT[:, 2 * ib:2 * ib + 1])
                    nc.vector.tensor_add(out_sb[:], out_sb[:], bv_bcast[:])
                    r0 = g * 512 + ib * P
                    e = (nc.sync, nc.gpsimd, nc.scalar)[ib % 3]
                    e.dma_start(out[r0:r0 + P, :], out_sb[:])

    nc.compile()
    return nc


def make_in_maps(x, Wq, bq, Wk, bk, Wv, bv):
    x = np.asarray(x, np.float32)
    wqT = np.asarray(Wq, np.float32).T.astype(BF16)
    wkT = np.asarray(Wk, np.float32).T.astype(BF16)
    wvT = np.asarray(Wv, np.float32).T.astype(BF16)
    bq = np.ascontiguousarray(np.asarray(bq, np.float32))
    bk = np.ascontiguousarray(np.asarray(bk, np.float32))
    bv = np.ascontiguousarray(np.asarray(bv, np.float32))
    in_maps = []
    for c in range(NCORES):
        b, h = c // 2, c % 2
        xb = x[b]
        in_maps.append({
            "xkvT": xb.T.astype(BF16),
            "xqT": xb[h * NQ:(h + 1) * NQ].T.astype(BF16),
            "wqT": wqT, "wkT": wkT, "wvT": wvT,
            "bq": bq, "bk": bk, "bv": bv,
        })
    return in_maps


def get_nc():
    if "nc" not in _CACHE:
        _CACHE["nc"] = build_nc()
    return _CACHE["nc"]


def kernel(x, Wq, bq, Wk, bk, Wv, bv):
    from concourse.bass_utils import run_bass_kernel_spmd
    nc = get_nc()
    in_maps = make_in_maps(x, Wq, bq, Wk, bk, Wv, bv)
    res = run_bass_kernel_spmd(nc, in_maps, core_ids=list(range(NCORES)))
    out = np.empty((B, S, D), np.float32)
    for c in range(NCORES):
        b, h = c // 2, c % 2
        out[b, h * NQ:(h + 1) * NQ] = res.results[c]["out"]
    return out


# revision 7
# speedup vs baseline: 1.3798x; 1.0205x over previous
"""Single-head attention (InterModalAttention) Bass kernel for 8 TRN2 cores.

Sharding: batch (4) x query-half (2) -> 8 cores. Each core computes K/V for
its batch element (full 2048-seq) and attention for its 1024 queries.

v2 layout strategy (all matmuls contract over the partition dim, all matmul
operands bf16 -> full PE rate, half the DMA/SBUF of fp32):
  - Host pre-transposes x and weights and casts to bf16: xT [d,s], WT [d,e].
  - qT/kT computed as [e, i/j] tiles (lhsT=WT tile, rhs=xT tile); bias added
    per-partition during the PSUM->SBUF copyback. qT stays SBUF-resident
    (no DRAM round-trip).
  - v computed natural [j, e] (lhsT=xT tile, rhs=WvT tile); bv folded into
    the epilogue (softmax rows sum to 1). K and V projections share one
    streaming pass over xT.
  - scores computed TRANSPOSED [j, i] (lhsT=kT tile, rhs=qT tile) so the
    exp output is directly the lhsT of the attn@V matmul - no PE transposes.
  - rowsums as tiny attnT.T @ ones matmuls accumulating into PSUM [i, 1]
    columns, which is exactly the per-partition orientation the epilogue
    division needs.
  - epilogue: out = out_psum * (1/rowsum) + bv_bcast.
"""
import sys
import numpy as np

for p in ("/opt/trn_rl_repo",):
    if p not in sys.path:
        sys.path.insert(0, p)

import ml_dtypes

B, S, D = 4, 2048, 1024
NQ = 1024          # queries per core
NCORES = 8
P = 128
INV_SQRT_D = 1.0 / 32.0
BF16 = ml_dtypes.bfloat16

_CACHE = {}


def build_nc():
    from contextlib import ExitStack
    import concourse.mybir as mybir
    import concourse.tile as tile
    from concourse import bacc

    F32 = mybir.dt.float32
    BF = mybir.dt.bfloat16
    AF = mybir.ActivationFunctionType

    nc = bacc.Bacc("TRN2", debug=False)

    xkvT = nc.dram_tensor("xkvT", (D, S), BF, kind="ExternalInput")
    xqT = nc.dram_tensor("xqT", (D, NQ), BF, kind="ExternalInput")
    wqT = nc.dram_tensor("wqT", (D, D), BF, kind="ExternalInput")
    wkT = nc.dram_tensor("wkT", (D, D), BF, kind="ExternalInput")
    wvT = nc.dram_tensor("wvT", (D, D), BF, kind="ExternalInput")
    bq = nc.dram_tensor("bq", (D,), F32, kind="ExternalInput")
    bk = nc.dram_tensor("bk", (D,), F32, kind="ExternalInput")
    bv = nc.dram_tensor("bv", (D,), F32, kind="ExternalInput")
    out = nc.dram_tensor("out", (NQ, D), F32, kind="ExternalOutput")

    ET = D // P            # 8 e-tiles
    DT = D // P            # 8 d-tiles
    SC = S // 512          # 4 x stream chunks
    SB = S // P            # 16 j-tiles
    IG = NQ // 512         # 2 i-groups
    EC = D // 512          # 2 e-chunks

    with tile.TileContext(nc) as tc, ExitStack() as ctx:
        consts = ctx.enter_context(tc.tile_pool(name="consts", bufs=1))
        wpool = ctx.enter_context(tc.tile_pool(name="wpool", bufs=1))
        xqpool = ctx.enter_context(tc.tile_pool(name="xqpool", bufs=1))
        qpool = ctx.enter_context(tc.tile_pool(name="qpool", bufs=1))
        kpool = ctx.enter_context(tc.tile_pool(name="kpool", bufs=1))
        vpool = ctx.enter_context(tc.tile_pool(name="vpool", bufs=1))

        # ---- resident tensors ----
        wq_sb = wpool.tile([P, DT, D], BF)
        wk_sb = wpool.tile([P, DT, D], BF)
        wv_sb = wpool.tile([P, DT, D], BF)
        xq_sb = xqpool.tile([P, DT, NQ], BF)
        qT = qpool.tile([P, ET, NQ], BF)     # [e-part, e-tile, i]
        kT = kpool.tile([P, ET, S], BF)      # [e-part, e-tile, j]
        vN = vpool.tile([P, SB, D], BF)      # [j-part, j-tile, e]

        # ---- startup DMAs: wq on sync, xq on gpsimd in parallel so the
        # first Q-proj matmul can start as early as possible; everything
        # else behind them ----
        for dt in range(DT):
            nc.sync.dma_start(wq_sb[:, dt, :], wqT[dt * P:(dt + 1) * P, :])
            nc.gpsimd.dma_start(xq_sb[:, dt, :], xqT[dt * P:(dt + 1) * P, :])

        # constants on the scalar queue (tiny)
        bq_sb = consts.tile([P, ET], F32)
        nc.scalar.dma_start(bq_sb[:], bq[:].rearrange("(t p) -> p t", p=P))
        bk_sb = consts.tile([P, ET], F32)
        nc.scalar.dma_start(bk_sb[:], bk[:].rearrange("(t p) -> p t", p=P))
        bv_f = consts.tile([1, D], F32)
        nc.scalar.dma_start(bv_f[:], bv[:].rearrange("(one d) -> one d", one=1))
        ones_f = consts.tile([1, P], F32)
        nc.gpsimd.memset(ones_f[:], 1.0)
        ones_row = consts.tile([1, P], BF)
        nc.vector.tensor_copy(ones_row[:], ones_f[:])
        onesc_f = consts.tile([P, 1], F32)
        nc.gpsimd.memset(onesc_f[:], 1.0)
        ones_col = consts.tile([P, 1], BF)
        nc.vector.tensor_copy(ones_col[:], onesc_f[:])
        bv_bf = consts.tile([1, D], BF)
        nc.vector.tensor_copy(bv_bf[:], bv_f[:])

        bv_bcast = consts.tile([P, D], F32)

        # remaining weight loads (behind the startup DMAs, ahead of use)
        for dt in range(DT):
            nc.sync.dma_start(wk_sb[:, dt, :], wkT[dt * P:(dt + 1) * P, :])
        for dt in range(DT):
            nc.scalar.dma_start(wv_sb[:, dt, :], wvT[dt * P:(dt + 1) * P, :])

        # ---- Phase 1: Q projection -> qT resident [e, i] ----
        # dt-outer across all 8 PSUM banks: the first matmul only needs the
        # first dt-row of wq and xq, so the PE starts ~1.5us into the kernel
        # and DMA pipelines the rest.
        with tc.tile_pool(name="qps", bufs=1, space="PSUM") as qps:
            # bv broadcast to [P, D] via ones.T @ bv (K=1 matmul)
            for ec in range(EC):
                pstmp = qps.tile([P, 512], F32, tag=f"q{ec}", name=f"pstmp{ec}")
                nc.tensor.matmul(pstmp[:], ones_row[:], bv_bf[:, ec * 512:(ec + 1) * 512],
                                 start=True, stop=True)
                nc.any.tensor_copy(bv_bcast[:, ec * 512:(ec + 1) * 512], pstmp[:])
            for g in range(IG):
                psqs = [qps.tile([P, 512], F32, tag=f"q{et}", name=f"psq{et}")
                        for et in range(ET)]
                for dt in range(DT):
                    for et in range(ET):
                        nc.tensor.matmul(psqs[et][:], wq_sb[:, dt, et * P:(et + 1) * P],
                                         xq_sb[:, dt, g * 512:(g + 1) * 512],
                                         start=(dt == 0), stop=(dt == DT - 1))
                for et in range(ET):
                    nc.vector.tensor_scalar_add(qT[:, et, g * 512:(g + 1) * 512],
                                                psqs[et][:], bq_sb[:, et:et + 1])

        # ---- Phase 2: K + V projections on one streaming pass of xkvT ----
        with tc.tile_pool(name="ps512p", bufs=2, space="PSUM") as ps512, \
             tc.tile_pool(name="xkv", bufs=2) as xkvp:
            for sc in range(SC):
                xg = xkvp.tile([P, DT, 512], BF, tag="xkv")
                for dt in range(DT):
                    e = (nc.sync, nc.gpsimd, nc.scalar)[dt % 3]
                    e.dma_start(xg[:, dt, :],
                                xkvT[dt * P:(dt + 1) * P, sc * 512:(sc + 1) * 512])
                for et in range(ET):
                    psk = ps512.tile([P, 512], F32, tag="ps512")
                    for dt in range(DT):
                        nc.tensor.matmul(psk[:], wk_sb[:, dt, et * P:(et + 1) * P],
                                         xg[:, dt, :], start=(dt == 0), stop=(dt == DT - 1))
                    nc.vector.tensor_scalar_add(kT[:, et, sc * 512:(sc + 1) * 512],
                                                psk[:], bk_sb[:, et:et + 1])
                for j4 in range(4):
                    jg = sc * 4 + j4
                    for ec in range(EC):
                        psv = ps512.tile([P, 512], F32, tag="ps512")
                        for dt in range(DT):
                            nc.tensor.matmul(psv[:], xg[:, dt, j4 * P:(j4 + 1) * P],
                                             wv_sb[:, dt, ec * 512:(ec + 1) * 512],
                                             start=(dt == 0), stop=(dt == DT - 1))
                        nc.any.tensor_copy(vN[:, jg, ec * 512:(ec + 1) * 512], psv[:])

        # ---- Phase 3: attention ----
        with tc.tile_pool(name="sps", bufs=2, space="PSUM") as ps512, \
             tc.tile_pool(name="outps", bufs=2, space="PSUM") as outps, \
             tc.tile_pool(name="rsps", bufs=1, space="PSUM") as rsps, \
             tc.tile_pool(name="attn", bufs=1) as attnp, \
             tc.tile_pool(name="epi", bufs=2) as epip:
            for g in range(IG):
                aT = attnp.tile([P, SB, 512], BF, tag="attn")  # [j-part, j-tile, i]
                # stride-2 columns: one sequential accumulation group per
                # column, 8-byte-aligned PSUM writes
                rs_ps = rsps.tile([P, 8], F32, tag="rs")       # [i-part, 2*i-block]
                for jt in range(SB):
                    sps = ps512.tile([P, 512], F32, tag="ps512")
                    for et in range(ET):
                        nc.tensor.matmul(sps[:], kT[:, et, jt * P:(jt + 1) * P],
                                         qT[:, et, g * 512:(g + 1) * 512],
                                         start=(et == 0), stop=(et == ET - 1))
                    nc.scalar.activation(aT[:, jt, :], sps[:], AF.Exp,
                                         scale=INV_SQRT_D)
                invT = epip.tile([P, 8], F32, tag="invT")
                for ib in range(4):
                    out_ps = [outps.tile([P, 512], F32, tag=f"outps{ec}", name=f"out_ps{ec}")
                              for ec in range(EC)]
                    for jg in range(SB):
                        for ec in range(EC):
                            nc.tensor.matmul(out_ps[ec][:],
                                             aT[:, jg, ib * P:(ib + 1) * P],
                                             vN[:, jg, ec * 512:(ec + 1) * 512],
                                             start=(jg == 0), stop=(jg == SB - 1))
                        # rowsum for this (ib, jg) chunk: reads the aT chunk
                        # the matmuls above just consumed, so exp is done;
                        # per-column group is sequential within the rs bank
                        nc.tensor.matmul(rs_ps[:, 2 * ib:2 * ib + 1],
                                         aT[:, jg, ib * P:(ib + 1) * P],
                                         ones_col[:],
                                         start=(jg == 0), stop=(jg == SB - 1))
                    nc.vector.reciprocal(invT[:, 2 * ib:2 * ib + 1],
                                         rs_ps[:, 2 * ib:2 * ib + 1])
                    out_sb = epip.tile([P, D], F32, tag="out_sb")
                    for ec in range(EC):
                        nc.vector.tensor_scalar_mul(out_sb[:, ec * 512:(ec + 1) * 512],
                                                    out_ps[ec][:], invT[:, 2 * ib:2 * ib + 1])
                    nc.vector.tensor_add(out_sb[:], out_sb[:], bv_bcast[:])
                    r0 = g * 512 + ib * P
                    e = (nc.sync, nc.gpsimd, nc.scalar)[ib % 3]
                    e.dma_start(out[r0:r0 + P, :], out_sb[:])

    nc.compile()
    return nc


def make_in_maps(x, Wq, bq, Wk, bk, Wv, bv):
    x = np.asarray(x, np.float32)
    wqT = np.asarray(Wq, np.float32).T.astype(BF16)
    wkT = np.asarray(Wk, np.float32).T.astype(BF16)
    wvT = np.asarray(Wv, np.float32).T.astype(BF16)
    bq = np.ascontiguousarray(np.asarray(bq, np.float32))
    bk = np.ascontiguousarray(np.asarray(bk, np.float32))
    bv = np.ascontiguousarray(np.asarray(bv, np.float32))
    in_maps = []
    for c in range(NCORES):
        b, h = c // 2, c % 2
        xb = x[b]
        in_maps.append({
            "xkvT": xb.T.astype(BF16),
            "xqT": xb[h * NQ:(h + 1) * NQ].T.astype(BF16),
            "wqT": wqT, "wkT": wkT, "wvT": wvT,
            "bq": bq, "bk": bk, "bv": bv,
        })
    return in_maps


def get_nc():
    if "nc" not in _CACHE:
        _CACHE["nc"] = build_nc()
    return _CACHE["nc"]


def kernel(x, Wq, bq, Wk, bk, Wv, bv):
    from concourse.bass_utils import run_bass_kernel_spmd
    nc = get_nc()
    in_maps = make_in_maps(x, Wq, bq, Wk, bk, Wv, bv)
    res = run_bass_kernel_spmd(nc, in_maps, core_ids=list(range(NCORES)))
    out = np.empty((B, S, D), np.float32)
    for c in range(NCORES):
        b, h = c // 2, c % 2
        out[b, h * NQ:(h + 1) * NQ] = res.results[c]["out"]
    return out
